# revision 11
# baseline (speedup 1.0000x reference)
"""Trainium2 Bass kernel for nn_ActorCritic_25013889532574 (loss_fn).

Computes (critic_loss, actor_loss) for an actor-critic loss with a
discounted-return scan, normalization stats over a random index subset,
and indexed loss sums — matching the oracle's exact semantics.

Oracle semantics (established by the validated v1 kernel)
---------------------------------------------------------
The reference's reverse associative scan computes G_t whose reversed-time
form u = T-1-t is the plain prefix sum of z_u = gamma^u * r_rev[u]. In
float32, gamma^u underflows to exactly 0 for u > ~10.4k, so G is a short
ramp on the first HEAD=16384 reversed positions followed by an exactly
constant plateau C = sum_j gamma^j r_rev[j]. Writing G = C + Delta
(Delta supported on u < HEAD) and beta = C - mean = -D1/n, every indexed
reduction becomes a combination of
  * full-index-set sums   T1=sum w, T2=sum w v, T3=sum w v^2,
                          T4=sum c lp, T5=sum c lp v, T6=sum c e
  * tiny head-region sums D1=sum c D, D2=sum c D^2, D3=sum w D,
                          D4=sum w D^2, D5=sum w D v, D6=sum c lp D
with c = include-multiplicity counts and w = c * is_random, giving
      var    = (D2 + 2 beta D1 + beta^2 n) / (n-1),  s = sqrt(var)+EPS
      critic = (D4 + 2 beta D3 + beta^2 T1)/s^2 - 2 (D5 + beta T2)/s + T3
      actor  = -(D6 + beta T4)/s + T5 - ALPHA T6

Expansion strategy (v2)
-----------------------
Positions never indexed by `to_include` contribute nothing to any sum, so
the host expands by multiplicity: it gathers v/lp/e at the `to_include`
indices (the same gather the reference itself performs) and partitions
the expanded stream by the is_random mask into group1 (mask=1) and
group0 (mask=0). Then
  T1 = |group1| (host integer),          T2 = sum v1,   T3 = sum v1^2,
  T4 = sum lp1 + sum lp0,  T5 = sum lp1 v1 + sum lp0 v0,
  T6 = sum e1 + sum e0
i.e. plain sums/dot-products over ~4M f16 elements with no count arrays.
f16 (not bf16): the 11-bit mantissa keeps the T5 rounding error ~1 abs
(bf16's 8-bit gave +14 on an actor of ~973 — too close to the 2e-2 gate).

Engine assignment (v3, from hardware microbenchmarks)
-----------------------------------------------------
Measured: fused DVE accumulate ops (scalar_tensor_tensor / tensor_scalar
with accum_out) always run at 1x; tensor_tensor_reduce crashes the
runtime; PE column-sum matmuls (stationary ones weights, psum
accumulation) cost ~0.4-0.8ns/col and run on an otherwise idle engine.
So per core:
  DVE  T5 products (fused stt 1x, no alternative) + head/ramp pass
  ACT  T3 = Square(v1) accumulate + psum collapses + ramp-pass copies
  PE   T2/T4/T6 column sums -> 3 PSUM banks, ACT collapses them
  DMA  6 transfers total: 4 combined [v|lp|e] stream chunks on the sync
       ring, combined head block + output on the scalar ring (two HWDGE
       rings run in parallel; v2's 20 small DMAs serialized ~600ns each
       on the sync sequencer and cost ~7us of ramp-in).
Head/ramp pass unchanged from the validated baseline: one 16k f32 prefix
scan builds Delta, six multiply-accumulates produce D1..D6, gamma-powers
zeroed on cores 1..7 keep the SPMD graph uniform. Per-partition
accumulator columns go out raw (128 x 15 f32); the host does the final
fold in f64.
"""

import math

import numpy as np

T = 8388608
NCORES = 8
P = 128
HEAD = 16384  # gamma^u support: f32 gamma^u == 0 for u > ~10.4k
HF = HEAD // P  # 128 columns in head layout
GAMMA = 0.99
ALPHA = 0.01
EPS = 1e-8

COLGRAN = 1024  # F granularity: chunk sizes stay 512-multiples for PE
W = 512  # PE colsum sub-block width (= PSUM bank capacity in f32)


def _chunk_plan(F1: int, F0: int):
    """Chunks as (group, C, kind): kind 'pe' routes plain sums through the
    tensor engine's psum chains; the final small 'fused' chunk keeps its
    plain sums on DVE/ACT so the psum collapses overlap the stream tail."""
    g1 = [(1, F1 // 2, "pe"), (1, F1 - F1 // 2, "pe")]
    if F0 >= 1536:
        g0 = [(0, F0 - 1024, "pe"), (0, 512, "pe"), (0, 512, "fused")]
    else:
        g0 = [(0, F0 - 512, "pe"), (0, 512, "fused")]
    return g1 + g0
NHB = 7  # head blocks: hd, gvec, hc, hw, hv, hlp, ut

ND = 6  # D1..D6 head-region sums

_NC_CACHE = {}
LAST_RESULTS = None  # BassKernelResults of the most recent run (for profiling)


def _build_nc(F1: int, F0: int):
    import concourse.tile as tile
    from concourse import bacc, mybir

    f32 = mybir.dt.float32
    f16 = mybir.dt.float16
    mult = mybir.AluOpType.mult
    add = mybir.AluOpType.add
    Copy = mybir.ActivationFunctionType.Copy
    Identity = mybir.ActivationFunctionType.Identity
    Square = mybir.ActivationFunctionType.Square

    plan = _chunk_plan(F1, F0)
    nchunks = len(plan)

    nc = bacc.Bacc()

    s_d = [
        nc.declare_dram_parameter(f"sc{j}", [P * 3 * C], f16, isOutput=False)
        for j, (g, C, kind) in enumerate(plan)
    ]
    hd_d = nc.declare_dram_parameter("hdall", [P * NHB * HF], f32, isOutput=False)

    # accumulator columns: D1..D6 first, then dynamically assigned T columns.
    # colmap values are (col, is_pe): is_pe columns are replicated over all
    # 128 partitions by the ones-matmul and must be divided by P on the host.
    colmap = {k: [] for k in ("T2", "T3", "T4", "T5", "T6")}
    ncol = [ND]

    def newcol(name, is_pe):
        c = ncol[0]
        ncol[0] += 1
        colmap[name].append((c, is_pe))
        return c

    cT5 = [newcol("T5", False) for _ in range(nchunks)]
    cT3 = [newcol("T3", False) for _ in range(sum(1 for g, C, k in plan if g == 1))]
    cT2 = newcol("T2", True)
    cT4pe = newcol("T4", True)
    cT6pe = newcol("T6", True)
    cT4f = newcol("T4", False)
    cT6f = newcol("T6", False)
    NACC = ncol[0]

    out_d = nc.declare_dram_parameter("out", [P * NACC], f32, isOutput=True)

    from contextlib import ExitStack

    with tile.TileContext(nc) as tc, ExitStack() as ctx:
        consts = ctx.enter_context(tc.tile_pool(name="consts", bufs=1))
        inp = ctx.enter_context(tc.tile_pool(name="inp", bufs=1))
        prod = ctx.enter_context(tc.tile_pool(name="prod", bufs=1))
        small = ctx.enter_context(tc.tile_pool(name="small", bufs=1))
        psum = ctx.enter_context(tc.tile_pool(name="psum", bufs=1, space="PSUM"))

        st = []
        for j, (g, C, kind) in enumerate(plan):
            chunk_t = inp.tile([P, 3 * C], f16, tag=f"sc{j}")
            st.append(chunk_t)
        hd_all = small.tile([P, NHB * HF], f32, tag="hdall")

        # head DMA truly first on the sync ring: SDMA engines drain it
        # before any stream packet, so the support chain starts ~1us in.
        nc.sync.dma_start(hd_all[:], hd_d[:].rearrange("(p f) -> p f", p=P))
        for j in range(nchunks):
            nc.sync.dma_start(
                st[j][:], s_d[j][:].rearrange("(p f) -> p f", p=P)
            )

        def sv(j):  # v / lp / e views of a combined chunk tile
            C = plan[j][1]
            t = st[j]
            return t[:, 0:C], t[:, C : 2 * C], t[:, 2 * C : 3 * C]

        # head block views
        hd_t = hd_all[:, 0 * HF : 1 * HF]
        gv_t = hd_all[:, 1 * HF : 2 * HF]
        hc_t = hd_all[:, 2 * HF : 3 * HF]
        hw_t = hd_all[:, 3 * HF : 4 * HF]
        hv_t = hd_all[:, 4 * HF : 5 * HF]
        hlp_t = hd_all[:, 5 * HF : 6 * HF]
        ut_t = hd_all[:, 6 * HF : 7 * HF]

        ones32 = consts.tile([P, P], f32)
        nc.vector.memset(ones32[:], 1.0)
        ones16 = consts.tile([P, P], f16)
        nc.vector.memset(ones16[:], 1.0)

        Cmax = max(C for g, C, kind in plan)
        acc = small.tile([P, NACC], f32, tag="acc")
        tr_v = prod.tile([P, Cmax], f16, tag="trv")  # DVE scratch
        tr_a = prod.tile([P, Cmax], f16, tag="tra")  # ACT scratch
        tr_c = prod.tile([P, W], f32, tag="trc")  # ACT collapse scratch

        # support-pass tiles
        zh = small.tile([P, HF], f32, tag="zh")
        ajunk = small.tile([P, HF], f32, tag="ajunk")
        rowsum = small.tile([P, 1], f32, tag="rowsum")
        pf_col = small.tile([P, 1], f32, tag="pfcol")
        ncs_col = small.tile([P, 1], f32, tag="ncscol")
        ramp = small.tile([P, HF], f32, tag="ramp")
        delta = small.tile([P, HF], f32, tag="delta")
        cd = small.tile([P, HF], f32, tag="cd")
        wd = small.tile([P, HF], f32, tag="wd")
        htr = small.tile([P, HF], f32, tag="htr")

        pf_ps = psum.tile([P, 1], f32, tag="pfps")
        cs_ps = psum.tile([P, 1], f32, tag="csps")
        ps2 = psum.tile([P, W], f32, tag="ps2")
        ps4 = psum.tile([P, W], f32, tag="ps4")
        ps6 = psum.tile([P, W], f32, tag="ps6")

        def stt(out_t, in0, in1, col):
            nc.vector.scalar_tensor_tensor(
                out_t, in0, 1.0, in1, mult, mult, accum_out=acc[:, col : col + 1]
            )

        def ts_sum(out_t, in0, col):
            nc.vector.tensor_scalar(
                out_t, in0, 1.0, 0.0, mult, add,
                accum_out=acc[:, col : col + 1],
            )

        # PE colsum chains over the 'pe' chunks only
        chain_total = {
            "ps2": sum(C // W for g, C, kind in plan if g == 1 and kind == "pe"),
            "ps4": sum(C // W for g, C, kind in plan if kind == "pe"),
            "ps6": sum(C // W for g, C, kind in plan if kind == "pe"),
        }
        chain_done = {"ps2": 0, "ps4": 0, "ps6": 0}
        ps_tiles = {"ps2": ps2, "ps4": ps4, "ps6": ps6}

        def colsum(name, view, C):
            t = ps_tiles[name]
            for off in range(0, C, W):
                first = chain_done[name] == 0
                chain_done[name] += 1
                last = chain_done[name] == chain_total[name]
                nc.tensor.matmul(
                    t[:], ones16[:], view[:, off : off + W],
                    start=first, stop=last,
                )

        # ---------- support chain first (head lands ~1us in) ----------
        nc.vector.tensor_mul(zh[:], hd_t, gv_t)
        nc.scalar.activation(ajunk[:], zh[:], Copy, accum_out=rowsum[:])
        nc.tensor.matmul(pf_ps[:], ut_t, rowsum[:, 0:1], start=True, stop=True)
        nc.tensor.matmul(cs_ps[:], ones32[:], rowsum[:, 0:1], start=True, stop=True)
        nc.scalar.activation(pf_col[:], pf_ps[:], Copy)
        nc.scalar.activation(ncs_col[:], cs_ps[:], Copy, scale=-1.0)
        nc.vector.tensor_tensor_scan(
            ramp[:], ones32[:, 0:HF], zh[:], pf_col[:, 0:1], mult, add
        )
        nc.scalar.activation(delta[:], ramp[:], Identity, bias=ncs_col[:, 0:1])

        # D-sums on DVE
        stt(cd[:], hc_t, delta[:], 0)  # D1 = sum c*Delta
        stt(htr[:], cd[:], delta[:], 1)  # D2 = sum c*Delta^2
        stt(wd[:], hw_t, delta[:], 2)  # D3 = sum w*Delta
        stt(htr[:], wd[:], delta[:], 3)  # D4 = sum w*Delta^2
        stt(htr[:], wd[:], hv_t, 4)  # D5 = sum w*Delta*v
        stt(htr[:], cd[:], hlp_t, 5)  # D6 = sum c*lp*Delta

        # ---------- main streaming ops in DMA arrival order ----------
        it3 = 0
        for j, (g, C, kind) in enumerate(plan):
            v, lp, e = sv(j)
            stt(tr_v[:, 0:C], lp, v, cT5[j])  # DVE: T5
            if g == 1:
                nc.scalar.activation(
                    tr_a[:, 0:C], v, Square,
                    accum_out=acc[:, cT3[it3] : cT3[it3] + 1],
                )
                it3 += 1
            if kind == "pe":
                if g == 1:
                    colsum("ps2", v, C)
                colsum("ps4", lp, C)
                colsum("ps6", e, C)
            else:  # fused tail chunk: keep the psum chains closed earlier
                ts_sum(tr_v[:, 0:C], lp, cT4f)
                nc.scalar.activation(
                    tr_a[:, 0:C], e, Copy, accum_out=acc[:, cT6f : cT6f + 1]
                )

        # collapse PE psum banks into acc columns (chains close before the
        # fused tail chunk arrives, so these overlap the stream)
        nc.scalar.activation(tr_c[:], ps2[:], Copy, accum_out=acc[:, cT2 : cT2 + 1])
        nc.scalar.activation(tr_c[:], ps4[:], Copy, accum_out=acc[:, cT4pe : cT4pe + 1])
        nc.scalar.activation(tr_c[:], ps6[:], Copy, accum_out=acc[:, cT6pe : cT6pe + 1])

        nc.sync.dma_start(out_d[:].rearrange("(p f) -> p f", p=P), acc[:])

    if not nc.is_finalized():
        nc.finalize()
    return nc, colmap, NACC


def _get_nc(F1: int, F0: int):
    key = (F1, F0)
    if key not in _NC_CACHE:
        _NC_CACHE[key] = _build_nc(F1, F0)
    return _NC_CACHE[key]


def _pad_cols(nelem: int) -> int:
    percore = -(-max(nelem, 1) // NCORES)
    F = -(-percore // P)
    return max(COLGRAN, -(-F // COLGRAN) * COLGRAN)


def kernel(**inputs) -> np.ndarray:
    from concourse.bass_utils import run_bass_kernel_spmd

    f16 = np.float16

    r = np.ascontiguousarray(np.asarray(inputs["rewards"]), dtype=np.float32)
    v = np.ascontiguousarray(np.asarray(inputs["value_estimates"]), dtype=np.float32)
    lp = np.ascontiguousarray(np.asarray(inputs["log_probs"]), dtype=np.float32)
    e = np.ascontiguousarray(np.asarray(inputs["entropies"]), dtype=np.float32)
    ti = np.asarray(inputs["to_include"]).astype(np.int64).ravel()
    mk = np.asarray(inputs["is_random"]).astype(bool)

    assert r.shape == (T,), r.shape
    n = ti.shape[0]

    # Expand by multiplicity and partition by the is_random mask.
    m_at = mk[ti]
    idx1 = ti[m_at]
    idx0 = ti[~m_at]
    n1 = int(idx1.size)

    F1 = _pad_cols(idx1.size)
    F0 = _pad_cols(idx0.size)

    def shards(idx, F):
        tot = NCORES * P * F
        pad = tot - idx.size
        out = {}
        for name, arr in (("v", v), ("lp", lp), ("e", e)):
            g = arr[idx].astype(f16)
            if pad:
                g = np.concatenate([g, np.zeros(pad, f16)])
            out[name] = g.reshape(NCORES, P, F)
        return out

    s1 = shards(idx1, F1)
    s0 = shards(idx0, F0)

    plan = _chunk_plan(F1, F0)
    goff = {1: 0, 0: 0}
    chunk_slices = []  # (group, col slice) per chunk, in plan order
    for g, C, kind in plan:
        chunk_slices.append((g, slice(goff[g], goff[g] + C)))
        goff[g] += C

    def combined(j, i):
        g, cs = chunk_slices[j]
        s = s1 if g == 1 else s0
        return np.ascontiguousarray(
            np.concatenate([s["v"][i, :, cs], s["lp"][i, :, cs], s["e"][i, :, cs]], 1)
        ).ravel()

    # Head-region blocks in reversed time u = T-1-t (first HEAD entries).
    rrev = r[::-1]
    hd = rrev[:HEAD].reshape(P, HF)
    gvec = (
        np.exp(np.arange(HEAD, dtype=np.float64) * math.log(GAMMA))
        .astype(np.float32)
        .reshape(P, HF)
    )
    hsel = ti >= (T - HEAD)
    hu = (T - 1 - ti[hsel]).astype(np.int64)
    hc = np.bincount(hu, minlength=HEAD)[:HEAD].astype(np.float32)
    mkrev = mk[::-1][:HEAD]
    hw = np.where(mkrev, hc, 0.0).astype(np.float32).reshape(P, HF)
    hc = hc.reshape(P, HF)
    hv = v[::-1][:HEAD].reshape(P, HF)
    hlp = lp[::-1][:HEAD].reshape(P, HF)
    ut = np.triu(np.ones((P, P), np.float32), k=1)

    def head_all(i):
        gv = gvec if i == 0 else np.zeros((P, HF), np.float32)
        return np.ascontiguousarray(
            np.concatenate([hd, gv, hc, hw, hv, hlp, ut], axis=1).astype(np.float32)
        ).ravel()

    nc, colmap, NACC = _get_nc(F1, F0)

    in_maps = []
    for i in range(NCORES):
        m = {f"sc{j}": combined(j, i) for j in range(len(plan))}
        m["hdall"] = head_all(i)
        in_maps.append(m)

    import time as _time

    last_err = None
    for _attempt in range(4):
        try:
            res = run_bass_kernel_spmd(nc, in_maps, core_ids=list(range(NCORES)))
            break
        except Exception as err:  # wedged accelerator from a prior crash: retry
            last_err = err
            _time.sleep(3.0)
    else:
        raise last_err
    global LAST_RESULTS
    LAST_RESULTS = res

    colsum = np.zeros(NACC, np.float64)
    for i in range(NCORES):
        colsum += (
            np.asarray(res.results[i]["out"], dtype=np.float64)
            .reshape(P, NACC)
            .sum(axis=0)
        )

    D1, D2, D3, D4, D5, D6 = colsum[0:ND]

    # PE colsum chains replicate the total across all 128 partitions, so the
    # partition fold overcounts those columns by exactly P.
    def fold(name):
        return sum(colsum[c] / (P if is_pe else 1) for c, is_pe in colmap[name])

    T2, T3, T4, T5, T6 = (fold(k) for k in ("T2", "T3", "T4", "T5", "T6"))

    nf = float(n)
    beta = -D1 / nf
    var = (D2 + 2.0 * beta * D1 + beta * beta * nf) / (nf - 1.0)
    s = math.sqrt(max(var, 0.0)) + EPS
    critic = (
        (D4 + 2.0 * beta * D3 + beta * beta * n1) / (s * s)
        - 2.0 * (D5 + beta * T2) / s
        + T3
    )
    actor = -(D6 + beta * T4) / s + T5 - ALPHA * T6
    return np.array([critic, actor], dtype=np.float32)
